# revision 51
# baseline (speedup 1.0000x reference)
"""Trainium2 Bass kernel for nn_DTFDynamicLayer (moe_routing).

Self-contained: takes FULL inputs, returns FULL output. Two SPMD NEFFs on 8
NeuronCores:
  NEFF1 (scoring): token-parallel router scores (fp32 — top-k boundaries are
    ~6e-4 apart, bf16 would flip selections).
  Host: top-k per batch row (argpartition), gather selected tokens.
  NEFF2 (decoder): Qwen2 block on the T=B*k=2048 selected tokens.
    Attention is head-parallel (2 Q heads + 1 KV head per core); the o-proj
    full-D partial is ReduceScattered over the TOKEN axis (the only
    collective, 1MB out), after which each core owns 256 tokens and runs the
    ENTIRE MLP for them locally (full gate/up/down weights streamed from
    HBM under the matmuls, P kept in SBUF) — down-proj output is final, no
    second collective. RMS ln weights folded into the weight slices; rms r1
    and rope tables folded host-side; r2 computed on device.
  Host: scatter-add gated delta into a copy of hidden_states.
"""
import sys
sys.path.insert(0, "/opt/trn_rl_repo")
import math
import numpy as np
import ml_dtypes

import jax
from jax.sharding import Mesh, PartitionSpec
from jax.experimental.shard_map import shard_map

import concourse.bacc as bacc
import concourse.mybir as mybir
import concourse.tile as tile
from concourse.bass_utils import run_bass_kernel_spmd
from concourse.bass2jax import (_bass_exec_p, partition_id_tensor,
                                install_neuronx_cc_hook)


def _make_runner(nc, n_cores=8):
    """Persistent jitted shard_map executor for an SPMD Bass program."""
    install_neuronx_cc_hook()
    pname = nc.partition_id_tensor.name if nc.partition_id_tensor else None
    in_names, out_names, out_avals, zero_outs = [], [], [], []
    for alloc in nc.m.functions[0].allocations:
        if not isinstance(alloc, mybir.MemoryLocationSet):
            continue
        name = alloc.memorylocations[0].name
        if alloc.kind == "ExternalInput":
            if name != pname:
                in_names.append(name)
        elif alloc.kind == "ExternalOutput":
            shape = tuple(alloc.tensor_shape)
            dtype = mybir.dt.np(alloc.dtype)
            out_names.append(name)
            out_avals.append(jax.core.ShapedArray(shape, dtype))
            zero_outs.append(np.zeros(shape, dtype))
    all_in = list(in_names) + list(out_names)
    if pname is not None:
        all_in.append(pname)

    def _body(*args):
        operands = list(args)
        if pname is not None:
            operands.append(partition_id_tensor())
        return tuple(_bass_exec_p.bind(
            *operands, out_avals=tuple(out_avals), in_names=tuple(all_in),
            out_names=tuple(out_names), lowering_input_output_aliases=(),
            sim_require_finite=True, sim_require_nnan=True, nc=nc))

    devices = jax.devices()[:n_cores]
    mesh = Mesh(np.asarray(devices), ("core",))
    nin = len(in_names) + len(out_avals)
    sharded = jax.jit(
        shard_map(_body, mesh=mesh,
                  in_specs=(PartitionSpec("core"),) * nin,
                  out_specs=(PartitionSpec("core"),) * len(out_avals),
                  check_rep=False),
        keep_unused=True)
    concat_zeros = [np.zeros((n_cores * z.shape[0], *z.shape[1:]), z.dtype)
                    for z in zero_outs]

    def run(in_maps):
        concat_in = [np.concatenate([np.asarray(in_maps[c][nm])
                                     for c in range(n_cores)], axis=0)
                     for nm in in_names]
        outs = sharded(*concat_in, *concat_zeros)
        return [{nm: np.asarray(outs[i]).reshape(n_cores, *out_avals[i].shape)[c]
                 for i, nm in enumerate(out_names)}
                for c in range(n_cores)]

    return run

N_CORES = 8
B, S = 4, 4096
T = 2048
D = 2048
HD = 128
DFF = 8192
NK = D // 128            # 16 D-chunks
TC = T // 512            # 4 q-chunks
NTOK = T // 128          # 16 token blocks
TG = T // N_CORES        # 256 tokens per core for the MLP
NM = DFF // 128          # 64 dff m-tiles
TOKC = B * S // N_CORES  # scoring tokens per core
NCH = TOKC // 128
EPS = 1e-6
SM_SCALE = 1.0 / math.sqrt(HD)
ROPE_THETA = 10000.0

f32 = mybir.dt.float32
bf16 = mybir.dt.bfloat16
f8 = mybir.dt.float8e4
PM2 = mybir.MatmulPerfMode.DoubleRow
WSCALE = 16.0
PSCALE = 4.0
AF = mybir.ActivationFunctionType
OP = mybir.AluOpType
BF = ml_dtypes.bfloat16

_cache = {}
BENCH_MODE = False
MLP_FP8 = True
DOWN_FP8 = True     # False: down-proj stays bf16 (lower error, a bit slower)
QKV_FP8 = True      # x + wq/wk/wv in fp8 with DoubleRow
ATT_FP8 = True      # e/v/ot/wo in fp8; PM2-paired j-blocks and heads
ABLATE = "full"     # bench-only: "full" | "no_rs" | "no_mlp" | "attn_only"


def _ext_in(nc, name, shape, dtype, reg):
    """ExternalInput normally; internal DRAM tensor in BENCH_MODE."""
    if BENCH_MODE:
        t = nc.dram_tensor(name, shape, dtype)
        reg.append(t)
        return t
    return nc.dram_tensor(name, shape, dtype, kind="ExternalInput")


def _bench_init(nc, tc, pool, reg):
    """Fill fake-input internal tensors with a small constant (bench mode)."""
    if not reg:
        return
    CW = 1024
    zt = pool.tile([128, CW], bf16, tag="benchz", name="benchinit")
    nc.vector.memset(zt[:], 0.01)
    ztf = pool.tile([128, CW], f32, tag="benchzf", name="benchinitf")
    nc.vector.memset(ztf[:], 0.01)
    zt8 = pool.tile([128, CW], f8, tag="benchz8", name="benchinit8")
    nc.vector.memset(zt8[:], 0.01)
    n = 0
    for t in reg:
        src_t = {f32: ztf, f8: zt8}.get(t.dtype, zt)
        rows, cols = (t.shape if len(t.shape) == 2 else (t.shape[0], 1))
        for r in range(0, rows, 128):
            rr = min(128, rows - r)
            for cstart in range(0, cols, CW):
                cc = min(CW, cols - cstart)
                eng = nc.sync if n % 2 == 0 else nc.scalar
                n += 1
                eng.dma_start(out=t[r:r + rr, cstart:cstart + cc],
                              in_=src_t[0:rr, 0:cc])


# ======================= NEFF1: scoring =======================
def build_scoring(reps=1):
    nc = bacc.Bacc("TRN2", target_bir_lowering=False, debug=False,
                   num_devices=N_CORES)
    breg = []
    orig = _ext_in(nc, "orig", [TOKC, D], f32, breg)
    dsur = _ext_in(nc, "dsur", [TOKC, D], f32, breg)
    wb = _ext_in(nc, "wb", [128, D], f32, breg)
    scores = nc.dram_tensor("scores", [TOKC], f32, kind="ExternalOutput")

    with tile.TileContext(nc) as tc:
        with tc.tile_pool(name="io", bufs=3) as io, \
             tc.tile_pool(name="scratch", bufs=2) as scratch, \
             tc.tile_pool(name="bench0", bufs=1) as bench0, \
             tc.tile_pool(name="acc", bufs=2) as accp:
            _bench_init(nc, tc, bench0, breg)
            for rep in range(reps):
                wb_t = accp.tile([128, D], f32, tag="wb", name=f"wb{rep}")
                nc.sync.dma_start(out=wb_t[:], in_=wb[:, :])
                ss = accp.tile([128, NCH], f32, tag="ss", name=f"ss{rep}")
                dot = accp.tile([128, NCH], f32, tag="dot", name=f"dot{rep}")

                for i in range(NCH):
                    o_t = io.tile([128, D], f32, tag="o", name=f"o{rep}_{i}")
                    d_t = io.tile([128, D], f32, tag="p", name=f"p{rep}_{i}")
                    sl = slice(i * 128, (i + 1) * 128)
                    nc.sync.dma_start(out=o_t[:], in_=orig[sl, :])
                    nc.scalar.dma_start(out=d_t[:], in_=dsur[sl, :])

                    sq_t = scratch.tile([128, D], bf16, tag="sq",
                                        name=f"sq{rep}_{i}")
                    nc.scalar.activation(sq_t[:], d_t[:], AF.Square,
                                         accum_out=ss[:, i:i + 1])
                    pr_t = scratch.tile([128, D], bf16, tag="pr",
                                        name=f"pr{rep}_{i}")
                    nc.vector.scalar_tensor_tensor(pr_t[:], o_t[:], 1.0, wb_t[:],
                                                   op0=OP.mult, op1=OP.mult,
                                                   accum_out=dot[:, i:i + 1])

                sc = accp.tile([128, NCH], f32, tag="sc", name=f"sc{rep}")
                nc.vector.scalar_tensor_tensor(sc[:], ss[:], 0.5 / D, dot[:],
                                               op0=OP.mult, op1=OP.add)
                nc.sync.dma_start(out=scores.rearrange("(n p) -> p n", p=128),
                                  in_=sc[:])
    nc.compile()
    return nc


# ======================= NEFF2: decoder =======================
def build_decoder(reps=1):
    nc = bacc.Bacc("TRN2", target_bir_lowering=False, debug=False,
                   num_devices=N_CORES)
    breg = []
    qdt = f8 if QKV_FP8 else bf16
    if QKV_FP8:
        xT = _ext_in(nc, "xT", [128, NK * T], f8, breg)
    else:
        xT = _ext_in(nc, "xT", [D, T], bf16, breg)
    xg_in = _ext_in(nc, "xg_in", [D, TG], bf16, breg)
    wq_in = _ext_in(nc, "wq_in", [128, NK * 256], qdt, breg)
    wk_in = _ext_in(nc, "wk_in", [128, NK * 128], qdt, breg)
    wv_in = _ext_in(nc, "wv_in", [128, NK * 128], qdt, breg)
    wo_in = _ext_in(nc, "wo_in", [128, 2 * D],
                    f8 if ATT_FP8 else bf16, breg)
    if MLP_FP8:
        # combined gate+up, grouped MG m-tiles per DMA: [m][g|u][k][128]
        wgu_in = _ext_in(nc, "wgu_in", [128, NM * 2 * NK * 128], f8, breg)
        wg_in = wu_in = None
    else:
        wgu_in = None
        wg_in = _ext_in(nc, "wg_in", [128, NM * NK * 128], bf16, breg)
        wu_in = _ext_in(nc, "wu_in", [128, NM * NK * 128], bf16, breg)
    ddt = f8 if (MLP_FP8 and DOWN_FP8) else bf16
    wd_in = _ext_in(nc, "wd_in", [128, NK * NM * 128], ddt, breg)
    cos_in = _ext_in(nc, "cos_in", [128, T], bf16, breg)
    sin_in = _ext_in(nc, "sin_in", [128, T], bf16, breg)
    r1c_in = _ext_in(nc, "r1c_in", [128, NTOK], f32, breg)
    gate_in = _ext_in(nc, "gate_in", [128, 2 * TG], bf16, breg)
    delta = nc.dram_tensor("delta", [D, TG], f32, kind="ExternalOutput")
    o_part = nc.dram_tensor("o_part", [2, N_CORES, D // 2, TG], bf16)
    o_red = nc.dram_tensor("o_red", [D, TG], bf16)
    RG = [list(range(N_CORES))]

    with tile.TileContext(nc) as tc:
        with tc.tile_pool(name="wres", bufs=1) as wres, \
             tc.tile_pool(name="xch", bufs=(1 if QKV_FP8 else NK)) as xch, \
             tc.tile_pool(name="att", bufs=1) as att, \
             tc.tile_pool(name="ws", bufs=2) as ws, \
             tc.tile_pool(name="rsc", bufs=2) as rsc, \
             tc.tile_pool(name="mlp", bufs=1) as mlp, \
             tc.tile_pool(name="sm", bufs=3) as sm, \
             tc.tile_pool(name="ev", bufs=2) as ev, \
             tc.tile_pool(name="bench0", bufs=1) as bench0, \
             tc.tile_pool(name="ps", bufs=2, space="PSUM") as ps, \
             tc.tile_pool(name="psden", bufs=1, space="PSUM") as psden, \
             tc.tile_pool(name="psg", bufs=2, space="PSUM") as psg:

            _bench_init(nc, tc, bench0, breg)
            for rep in range(reps):
                _decoder_body(nc, tc, rep, xT, xg_in, wq_in, wk_in, wv_in,
                              wo_in, wg_in, wu_in, wgu_in, wd_in, cos_in,
                              sin_in, r1c_in, gate_in, delta, o_part, o_red,
                              RG, wres, xch, att, ws, rsc, mlp, sm, ev,
                              ps, psden, psg)
    nc.compile()
    return nc


def _decoder_body(nc, tc, rep, xT, xg_in, wq_in, wk_in, wv_in, wo_in, wg_in,
                  wu_in, wgu_in, wd_in, cos_in, sin_in, r1c_in, gate_in,
                  delta, o_part, o_red, RG, wres, xch, att, ws, rsc, mlp, sm,
                  ev, ps, psden, psg):
    qdt = f8 if QKV_FP8 else bf16
    # ---- resident small tensors ----
    wq_sb = wres.tile([128, NK, 256], qdt, tag="wq", name=f"wq{rep}")
    nc.sync.dma_start(out=wq_sb[:], in_=wq_in[:, :])
    wk_sb = wres.tile([128, NK, 128], qdt, tag="wk", name=f"wk{rep}")
    nc.sync.dma_start(out=wk_sb[:], in_=wk_in[:, :])
    wv_sb = wres.tile([128, NK, 128], qdt, tag="wv", name=f"wv{rep}")
    nc.sync.dma_start(out=wv_sb[:], in_=wv_in[:, :])
    if ATT_FP8:
        wo_sb = wres.tile([128, 2, D], f8, tag="wo", name=f"wo{rep}")
    else:
        wo_sb = wres.tile([128, 2 * D], bf16, tag="wo", name=f"wo{rep}")
    nc.sync.dma_start(out=wo_sb[:], in_=wo_in[:, :])
    r1c_sb = wres.tile([128, NTOK], f32, tag="r1c", name=f"r1c{rep}")
    nc.sync.dma_start(out=r1c_sb[:], in_=r1c_in[:, :])
    gate_sb = wres.tile([128, 2, TG], bf16, tag="gateb",
                        name=f"gate{rep}")
    nc.sync.dma_start(out=gate_sb[:], in_=gate_in[:, :])
    ones_sb = wres.tile([128, 1], bf16, tag="ones", name=f"ones{rep}")
    nc.vector.memset(ones_sb[:], 1.0)
    if ATT_FP8:
        # dual-fp8 Ldweights needs the pair-dim step 16B-aligned
        ones8_sb = wres.tile([128, 2, 16], f8, tag="ones8",
                             name=f"ones8{rep}")
        nc.vector.memset(ones8_sb[:], 1.0)
    cos_sb = ws.tile([128, T], bf16, tag="wsa", name=f"cos{rep}")
    nc.sync.dma_start(out=cos_sb[:], in_=cos_in[:, :])
    sin_sb = ws.tile([128, T], bf16, tag="wsb", name=f"sin{rep}")
    nc.sync.dma_start(out=sin_sb[:], in_=sin_in[:, :])

    # ---- x (full T, for QKV); f8 single tile or bf16 chunks ----
    if QKV_FP8:
        x8 = xch.tile([128, NK, T], f8, tag="x", name=f"x8{rep}")
        nc.sync.dma_start(out=x8[:], in_=xT[:, :])
        def xsl(k, sl):
            return x8[:, k, sl]
    else:
        x_sb = []
        for k in range(NK):
            xk = xch.tile([128, T], bf16, tag="x", name=f"x{rep}_{k}")
            nc.sync.dma_start(out=xk[:], in_=xT[k * 128:(k + 1) * 128, :])
            x_sb.append(xk)
        def xsl(k, sl):
            return x_sb[k][:, sl]

    # ---- Q/K projections ----
    qt = [att.tile([128, T], bf16, tag=f"qo{h}", name=f"qraw{rep}{h}")
          for h in range(2)]
    kt = att.tile([128, T], bf16, tag="kr2", name=f"kraw{rep}")
    for h in range(2):
        for n in range(TC):
            acc = ps.tile([128, 512], f32, tag="acc", name=f"qa{rep}{h}{n}")
            nsl = slice(n * 512, (n + 1) * 512)
            if QKV_FP8:
                for i in range(NK // 2):
                    nc.tensor.matmul(
                        acc[:], wq_sb[:, 2 * i:2 * i + 2,
                                      h * 128:(h + 1) * 128],
                        x8[:, 2 * i:2 * i + 2, nsl],
                        start=(i == 0), stop=(i == NK // 2 - 1),
                        perf_mode=PM2)
            else:
                for k in range(NK):
                    nc.tensor.matmul(
                        acc[:], wq_sb[:, k, h * 128:(h + 1) * 128],
                        xsl(k, nsl), start=(k == 0), stop=(k == NK - 1))
            if n % 2 == 0:
                nc.scalar.copy(qt[h][:, nsl], acc[:])
            else:
                nc.vector.tensor_scalar(qt[h][:, nsl],
                                        acc[:], 1.0, None, op0=OP.mult)
    for n in range(TC):
        acc = ps.tile([128, 512], f32, tag="acc", name=f"ka{rep}{n}")
        nsl = slice(n * 512, (n + 1) * 512)
        if QKV_FP8:
            for i in range(NK // 2):
                nc.tensor.matmul(
                    acc[:], wk_sb[:, 2 * i:2 * i + 2, :],
                    x8[:, 2 * i:2 * i + 2, nsl],
                    start=(i == 0), stop=(i == NK // 2 - 1), perf_mode=PM2)
        else:
            for k in range(NK):
                nc.tensor.matmul(
                    acc[:], wk_sb[:, k, :], xsl(k, nsl),
                    start=(k == 0), stop=(k == NK - 1))
        if n % 2 == 0:
            nc.vector.tensor_scalar(kt[:, nsl],
                                    acc[:], 1.0, None, op0=OP.mult)
        else:
            nc.scalar.copy(kt[:, nsl], acc[:])

    # ---- V projection (r1 fused; token-partition layout) ----
    vdt = f8 if ATT_FP8 else bf16
    v_sb = att.tile([128, NTOK, 128], vdt, tag="vg", name=f"vg{rep}")
    for j in range(NTOK):
        acc = ps.tile([128, 512], f32, tag="acc", name=f"va{rep}{j}")
        jsl = slice(j * 128, (j + 1) * 128)
        if QKV_FP8:
            for i in range(NK // 2):
                nc.tensor.matmul(
                    acc[:, 0:128], x8[:, 2 * i:2 * i + 2, jsl],
                    wv_sb[:, 2 * i:2 * i + 2, :],
                    start=(i == 0), stop=(i == NK // 2 - 1), perf_mode=PM2)
        else:
            for k in range(NK):
                nc.tensor.matmul(
                    acc[:, 0:128], xsl(k, jsl), wv_sb[:, k, :],
                    start=(k == 0), stop=(k == NK - 1))
        nc.vector.tensor_scalar(
            v_sb[:, j, :], acc[:, 0:128],
            r1c_sb[:, j:j + 1], None, op0=OP.mult)

    # ---- rope (in place: qt/kt tiles become the roped values) ----
    for raw in (qt[0], qt[1], kt):
        for hh in range(2):
            csl = slice(hh * (T // 2), (hh + 1) * (T // 2))
            swp = rsc.tile([128, T // 2], bf16, tag="swp", bufs=2,
                           name=f"swp{rep}")
            nc.sync.dma_start(out=swp[0:64, :], in_=raw[64:128, csl])
            nc.sync.dma_start(out=swp[64:128, :], in_=raw[0:64, csl])
            t1 = rsc.tile([128, T // 2], bf16, tag="t1", bufs=2,
                          name=f"t1{rep}")
            nc.vector.tensor_tensor(t1[:], raw[:, csl], cos_sb[:, csl],
                                    op=OP.mult)
            nc.vector.tensor_tensor(swp[:], swp[:], sin_sb[:, csl],
                                    op=OP.mult)
            nc.vector.tensor_tensor(raw[:, csl], t1[:], swp[:], op=OP.add)

    # ---- attention -> per-chunk softmax outputs (persisted) ----
    # The two heads' j-chains are interleaved so one head's exp (Act engine)
    # hides under the other head's matmuls.
    sm_sc = SM_SCALE / (WSCALE * WSCALE) if QKV_FP8 else SM_SCALE
    ot_c = []
    for c in range(TC):
        nj = 4 * c + 4
        o_acc = [ps.tile([128, 512], f32, tag="oacc", bufs=2,
                         name=f"oa{rep}{h}{c}") for h in range(2)]
        den = [psden.tile([1, 512], f32, tag="den", bufs=2,
                          name=f"dn{rep}{h}{c}") for h in range(2)]
        if ATT_FP8:
            for jp in range(nj // 2):
                eps = [sm.tile([128, 2, 512], f8, tag=f"exp{h}", bufs=2,
                               name=f"e{rep}") for h in range(2)]
                for q01 in range(2):
                    j = 2 * jp + q01
                    for h in range(2):
                        s_ps = ps.tile([128, 512], f32, tag="acc",
                                       name=f"s{rep}{h}{c}{j}")
                        nc.tensor.matmul(
                            s_ps[:], kt[:, j * 128:(j + 1) * 128],
                            qt[h][:, c * 512:(c + 1) * 512],
                            start=True, stop=True)
                        if j >= 4 * c:
                            er = sm.tile([128, 512], f8, tag="expr", bufs=2,
                                         name=f"er{rep}")
                            nc.scalar.activation(er[:], s_ps[:], AF.Exp,
                                                 scale=sm_sc)
                            nc.gpsimd.affine_select(
                                eps[h][:, q01, :], er[:], pattern=[[1, 512]],
                                compare_op=OP.is_ge, fill=0.0,
                                base=-(j - 4 * c) * 128,
                                channel_multiplier=-1)
                        else:
                            nc.scalar.activation(eps[h][:, q01, :], s_ps[:],
                                                 AF.Exp, scale=sm_sc)
                for h in range(2):
                    nc.tensor.matmul(o_acc[h][:],
                                     v_sb[:, 2 * jp:2 * jp + 2, :],
                                     eps[h][:], start=(jp == 0),
                                     stop=(jp == nj // 2 - 1), perf_mode=PM2)
                    nc.tensor.matmul(den[h][:], ones8_sb[:, :, 0:1],
                                     eps[h][:], start=(jp == 0),
                                     stop=(jp == nj // 2 - 1), perf_mode=PM2)
            otp = ev.tile([128, 2, 512], f8, tag=f"otp{c}", name=f"ot{rep}{c}")
            for h in range(2):
                den_sb = ev.tile([1, 512], f32, tag="densb", name=f"dsb{rep}")
                nc.vector.reciprocal(den_sb[:], den[h][:])
                den_b = ev.tile([128, 512], f32, tag="denb", name=f"db{rep}")
                nc.gpsimd.partition_broadcast(den_b[:], den_sb[:])
                nc.vector.tensor_tensor(otp[:, h, :], o_acc[h][:], den_b[:],
                                        op=OP.mult)
            ot_c.append(otp)
        else:
            ot = [None, None]
            for j in range(nj):
                es = []
                for h in range(2):
                    s_ps = ps.tile([128, 512], f32, tag="acc",
                                   name=f"s{rep}{h}{c}{j}")
                    nc.tensor.matmul(
                        s_ps[:], kt[:, j * 128:(j + 1) * 128],
                        qt[h][:, c * 512:(c + 1) * 512],
                        start=True, stop=True)
                    e = sm.tile([128, 512], bf16, tag="exp", name=f"e{rep}")
                    nc.scalar.activation(e[:], s_ps[:], AF.Exp, scale=sm_sc)
                    if j >= 4 * c:
                        v_ = j - 4 * c
                        e2 = sm.tile([128, 512], bf16, tag="exp2", bufs=2,
                                     name=f"e2{rep}")
                        nc.gpsimd.affine_select(
                            e2[:], e[:], pattern=[[1, 512]],
                            compare_op=OP.is_ge, fill=0.0,
                            base=-v_ * 128, channel_multiplier=-1)
                        e = e2
                    es.append(e)
                for h in range(2):
                    nc.tensor.matmul(o_acc[h][:],
                                     v_sb[:, j * 128:(j + 1) * 128],
                                     es[h][:], start=(j == 0),
                                     stop=(j == nj - 1))
                    nc.tensor.matmul(den[h][:], ones_sb[:], es[h][:],
                                     start=(j == 0), stop=(j == nj - 1))
            for h in range(2):
                den_sb = ev.tile([1, 512], f32, tag="densb", name=f"dsb{rep}")
                nc.vector.reciprocal(den_sb[:], den[h][:])
                den_b = ev.tile([128, 512], f32, tag="denb", name=f"db{rep}")
                nc.gpsimd.partition_broadcast(den_b[:], den_sb[:])
                oth = ev.tile([128, 512], bf16, tag=f"ot{c}{h}",
                              name=f"ot{rep}{h}{c}")
                nc.vector.tensor_tensor(oth[:], o_acc[h][:], den_b[:],
                                        op=OP.mult)
                ot[h] = oth
            ot_c.append(ot)

    # ---- o-proj m-major in D-halves; each half ReduceScatters while the
    # ---- next half computes
    for half in range(2):
        for m in range(half * (NK // 2), (half + 1) * (NK // 2)):
            for c in range(TC):
                acc = psg.tile([128, 512], f32, tag="g", name=f"op{rep}{m}{c}")
                if ATT_FP8:
                    nc.tensor.matmul(
                        acc[:], wo_sb[:, :, m * 128:(m + 1) * 128],
                        ot_c[c][:], start=True, stop=True, perf_mode=PM2)
                else:
                    nc.tensor.matmul(acc[:], wo_sb[:, m * 128:(m + 1) * 128],
                                     ot_c[c][0][:], start=True, stop=False)
                    nc.tensor.matmul(
                        acc[:], wo_sb[:, D + m * 128: D + (m + 1) * 128],
                        ot_c[c][1][:], start=False, stop=True)
                ob = ev.tile([128, 512], bf16, tag="ob", bufs=4,
                             name=f"ob{rep}")
                osc = 1.0 / WSCALE if ATT_FP8 else 1.0
                nc.vector.tensor_scalar(ob[:], acc[:], osc, None, op0=OP.mult)
                e0, e1 = ((nc.sync, nc.scalar) if (m + c) % 2 == 0
                          else (nc.scalar, nc.sync))
                lm = m % (NK // 2)
                e0.dma_start(
                    out=o_part[half, 2 * c, lm * 128:(lm + 1) * 128, :],
                    in_=ob[:, 0:256])
                e1.dma_start(
                    out=o_part[half, 2 * c + 1, lm * 128:(lm + 1) * 128, :],
                    in_=ob[:, 256:512])
        if ABLATE != "no_rs":
            off = half * (D // 2)
            nc.gpsimd.collective_compute(
                "ReduceScatter", OP.add, replica_groups=RG,
                ins=[o_part[half, :, :, :]],
                outs=[o_red[off:off + D // 2, :]])

    if ABLATE == "attn_only":
        fin = mlp.tile([128, NK, TG], bf16, tag="or", name=f"fin{rep}")
        nc.sync.dma_start(out=fin[:],
                          in_=xg_in.rearrange("(k p) t -> p k t", p=128))
        for m2 in range(NK):
            nc.gpsimd.dma_start(out=delta[m2 * 128:(m2 + 1) * 128, :],
                                in_=fin[:, m2, :])
        return

    # ---- x1 = xg + o_red; r2; h2 = x1*r2 (quantized once) ----
    P_F8 = MLP_FP8 and DOWN_FP8
    if MLP_FP8:
        x1f8 = mlp.tile([128, NK, TG], f8, tag="x1f8", name=f"x1f8{rep}")
        xg1 = mlp.tile([128, NK, TG], bf16, tag="xg", name=f"xg{rep}")
        nc.sync.dma_start(out=xg1[:],
                          in_=xg_in.rearrange("(k p) t -> p k t", p=128))
        or1 = mlp.tile([128, NK, TG], bf16, tag="or", name=f"or{rep}")
        nc.scalar.dma_start(out=or1[:],
                            in_=o_red.rearrange("(k p) t -> p k t", p=128))
        nc.vector.tensor_tensor(xg1[:], xg1[:], or1[:], op=OP.add)   # x1
        if ABLATE == "no_mlp":
            for m2 in range(NK):
                nc.gpsimd.dma_start(out=delta[m2 * 128:(m2 + 1) * 128, :],
                                    in_=xg1[:, m2, :])
            return
        r2ps = psden.tile([1, TG], f32, tag="den", bufs=2, name=f"r2ps{rep}")
        for k in range(NK):
            sq = sm.tile([128, TG], bf16, tag="sq", bufs=2,
                         name=f"sq{rep}{k}")
            nc.scalar.activation(sq[:], xg1[:, k, :], AF.Square)
            nc.tensor.matmul(r2ps[:], ones_sb[:], sq[:],
                             start=(k == 0), stop=(k == NK - 1))
    else:
        x1_sb, or_sb = [], []
        for k in range(NK):
            xg = mlp.tile([128, TG], bf16, tag=f"xg{k}", name=f"xg{rep}{k}")
            nc.sync.dma_start(out=xg[:], in_=xg_in[k * 128:(k + 1) * 128, :])
            ork = mlp.tile([128, TG], bf16, tag=f"or{k}", name=f"or{rep}{k}")
            nc.scalar.dma_start(out=ork[:],
                                in_=o_red[k * 128:(k + 1) * 128, :])
            nc.vector.tensor_tensor(xg[:], xg[:], ork[:], op=OP.add)
            x1_sb.append(xg)
            or_sb.append(ork)
        r2ps = psden.tile([1, TG], f32, tag="den", bufs=2, name=f"r2ps{rep}")
        for k in range(NK):
            sq = sm.tile([128, TG], bf16, tag="sq", bufs=2,
                         name=f"sq{rep}{k}")
            nc.scalar.activation(sq[:], x1_sb[k][:], AF.Square)
            nc.tensor.matmul(r2ps[:], ones_sb[:], sq[:],
                             start=(k == 0), stop=(k == NK - 1))
    mn = ev.tile([1, TG], f32, tag="r2mn", name=f"r2mn{rep}")
    nc.vector.tensor_scalar(mn[:], r2ps[:], 1.0 / D, EPS,
                            op0=OP.mult, op1=OP.add)
    rc = ev.tile([1, TG], f32, tag="r2rc", name=f"r2rc{rep}")
    nc.vector.reciprocal(rc[:], mn[:])
    r2row = ev.tile([1, TG], bf16, tag="r2row", name=f"r2row{rep}")
    nc.scalar.activation(r2row[:], rc[:], AF.Sqrt)
    r2b = ev.tile([128, TG], bf16, tag="r2b", name=f"r2b{rep}")
    nc.gpsimd.partition_broadcast(r2b[:], r2row[:])
    if MLP_FP8:
        for k in range(NK):
            nc.vector.tensor_tensor(x1f8[:, k, :], xg1[:, k, :], r2b[:],
                                    op=OP.mult)

    # ---- gate/up for my tokens: full DFF, weights streamed; P in SBUF ----
    if P_F8:
        p_sb = [mlp.tile([128, NK, TG], f8, tag=f"p{jj}", name=f"p{rep}_{jj}")
                for jj in range(4)]
    else:
        p_sb = [mlp.tile([128, 8 * TG], bf16, tag=f"p{jj}",
                         name=f"p{rep}_{jj}") for jj in range(8)]
    if MLP_FP8:
        # per-m combined gate+up loads, rotated across all three DMA-capable
        # queues; deep ring so several MB of weights prefetch during the
        # attention/collective window. Pairs of m-tiles share PSUM tiles so
        # the silu / p-write epilogues run on 512-col tiles (per-op overhead
        # on Act/DVE is large).
        wgu_g = [None] * NM
        sz = 2 * NK * 128
        for g in range(NM):
            t = ws.tile([128, 2, NK, 128], f8, tag="wgu",
                        bufs=8, name=f"wgu{rep}{g}")
            eng = (nc.sync, nc.scalar, nc.gpsimd)[g % 3]
            eng.dma_start(out=t[:], in_=wgu_in[:, g * sz:(g + 1) * sz])
            wgu_g[g] = t
        for mg in range(NM // 2):
            m0 = 2 * mg
            gps = ps.tile([128, 2, TG], f32, tag="acc", name=f"g{rep}{mg}")
            ups = ps.tile([128, 2, TG], f32, tag="oacc", name=f"u{rep}{mg}")
            for j in range(2):
                gt = wgu_g[m0 + j]
                for i in range(NK // 2):
                    nc.tensor.matmul(gps[:, j, :], gt[:, 0, 2 * i:2 * i + 2, :],
                                     x1f8[:, 2 * i:2 * i + 2, :],
                                     start=(i == 0), stop=(i == NK // 2 - 1),
                                     perf_mode=PM2)
                for i in range(NK // 2):
                    nc.tensor.matmul(ups[:, j, :], gt[:, 1, 2 * i:2 * i + 2, :],
                                     x1f8[:, 2 * i:2 * i + 2, :],
                                     start=(i == 0), stop=(i == NK // 2 - 1),
                                     perf_mode=PM2)
            gact = sm.tile([128, 2, TG], bf16, tag="gact", bufs=2,
                           name=f"gact{rep}")
            nc.scalar.activation(gact[:], gps[:], AF.Silu, scale=1.0 / WSCALE)
            if P_F8:
                nc.vector.scalar_tensor_tensor(
                    p_sb[m0 // 16][:, m0 % 16:m0 % 16 + 2, :], ups[:],
                    PSCALE / WSCALE, gact[:], op0=OP.mult, op1=OP.mult)
            else:
                nc.vector.scalar_tensor_tensor(
                    p_sb[m0 // 8][:, (m0 % 8) * TG:(m0 % 8 + 2) * TG],
                    ups[:], 1.0 / WSCALE, gact[:], op0=OP.mult, op1=OP.mult)
    else:
        for m in range(NM):
            wg_m = ws.tile([128, NK * 128], bf16, tag="wsa", name=f"wg{rep}{m}")
            wu_m = ws.tile([128, NK * 128], bf16, tag="wsb", name=f"wu{rep}{m}")
            nc.scalar.dma_start(
                out=wg_m[:], in_=wg_in[:, m * NK * 128:(m + 1) * NK * 128])
            nc.scalar.dma_start(
                out=wu_m[:], in_=wu_in[:, m * NK * 128:(m + 1) * NK * 128])
            gps = psg.tile([128, TG], f32, tag="g", name=f"g{rep}{m}")
            ups = psg.tile([128, TG], f32, tag="g", name=f"u{rep}{m}")
            for k in range(NK):
                nc.tensor.matmul(gps[:], wg_m[:, k * 128:(k + 1) * 128],
                                 x1_sb[k][:], start=(k == 0),
                                 stop=(k == NK - 1))
            for k in range(NK):
                nc.tensor.matmul(ups[:], wu_m[:, k * 128:(k + 1) * 128],
                                 x1_sb[k][:], start=(k == 0),
                                 stop=(k == NK - 1))
            gsc = sm.tile([128, TG], bf16, tag="gsc", bufs=2, name=f"gsc{rep}")
            nc.vector.tensor_tensor(gsc[:], gps[:], r2b[:], op=OP.mult)
            gact = sm.tile([128, TG], bf16, tag="gact", bufs=2,
                           name=f"gact{rep}")
            nc.scalar.activation(gact[:], gsc[:], AF.Silu)
            usc = sm.tile([128, TG], bf16, tag="usc", bufs=2, name=f"usc{rep}")
            nc.vector.tensor_tensor(usc[:], ups[:], r2b[:], op=OP.mult)
            nc.vector.tensor_tensor(
                p_sb[m // 8][:, (m % 8) * TG:(m % 8 + 1) * TG],
                gact[:], usc[:], op=OP.mult)

    # ---- down for my tokens: full contraction, output is final ----
    wd_tags = ("qo0", "qo1", "kr2", "vg")
    if P_F8:
        for pg in range(NK // 2):
            m2 = 2 * pg
            wd_ch = []
            for q in range(4):
                wdt = att.tile([128, NM // 2, 128], f8, tag=wd_tags[q],
                               name=f"wd{rep}{m2}{q}")
                eng = (nc.gpsimd, nc.sync)[q % 2]
                eng.dma_start(
                    out=wdt[:],
                    in_=wd_in[:, (m2 * 2 + q) * (NM // 2) * 128:
                              (m2 * 2 + q + 1) * (NM // 2) * 128])
                wd_ch.append(wdt)
            acc = psg.tile([128, 2, TG], f32, tag="g", name=f"d{rep}{pg}")
            for j in range(2):
                for i in range(NM // 2):
                    k = 2 * i
                    nc.tensor.matmul(
                        acc[:, j, :],
                        wd_ch[2 * j + k // 32][:, (k % 32):(k % 32) + 2, :],
                        p_sb[k // 16][:, k % 16:k % 16 + 2, :],
                        start=(i == 0), stop=(i == NM // 2 - 1),
                        perf_mode=PM2)
            dsum = sm.tile([128, 2, TG], f32, tag="dsum", bufs=2,
                           name=f"ds{rep}")
            nc.vector.scalar_tensor_tensor(
                dsum[:], acc[:], 1.0 / (WSCALE * PSCALE),
                or1[:, m2:m2 + 2, :], op0=OP.mult, op1=OP.add)
            dout = sm.tile([128, 2, TG], f32, tag="dout", bufs=2,
                           name=f"do{rep}")
            nc.vector.tensor_tensor(dout[:], dsum[:], gate_sb[:], op=OP.mult)
            for j in range(2):
                nc.scalar.dma_start(
                    out=delta[(m2 + j) * 128:(m2 + j + 1) * 128, :],
                    in_=dout[:, j, :])
    else:
        for m2 in range(NK):
            wd_ch = []
            for q in range(4):
                wdt = att.tile([128, T], bf16, tag=wd_tags[q],
                               name=f"wd{rep}{m2}{q}")
                eng = nc.gpsimd if q % 2 == 0 else nc.scalar
                eng.dma_start(
                    out=wdt[:],
                    in_=wd_in[:, (m2 * 4 + q) * T:(m2 * 4 + q + 1) * T])
                wd_ch.append(wdt)
            acc = psg.tile([128, TG], f32, tag="g", name=f"d{rep}{m2}")
            for k in range(NM):
                nc.tensor.matmul(
                    acc[:],
                    wd_ch[k // 16][:, (k % 16) * 128:(k % 16 + 1) * 128],
                    p_sb[k // 8][:, (k % 8) * TG:(k % 8 + 1) * TG],
                    start=(k == 0), stop=(k == NM - 1))
            ormk = (or1[:, m2, :] if MLP_FP8 else or_sb[m2][:])
            dsum = sm.tile([128, TG], f32, tag="dsum", bufs=2, name=f"ds{rep}")
            nc.vector.tensor_tensor(dsum[:], acc[:], ormk, op=OP.add)
            dout = sm.tile([128, TG], f32, tag="dout", bufs=2, name=f"do{rep}")
            nc.vector.tensor_tensor(dout[:], dsum[:], gate_sb[:, 0, :],
                                    op=OP.mult)
            nc.scalar.dma_start(out=delta[m2 * 128:(m2 + 1) * 128, :],
                                in_=dout[:])


def _get(name, builder):
    if name not in _cache:
        _cache[name] = builder()
    return _cache[name]


def _tile_w(w, kchunks, mblocks=None):
    K, M = w.shape
    if mblocks is None:
        return np.ascontiguousarray(
            w.reshape(kchunks, 128, M).transpose(1, 0, 2)
            .reshape(128, kchunks * M))
    mb = M // mblocks
    return np.ascontiguousarray(
        w.reshape(kchunks, 128, mblocks, mb).transpose(1, 2, 0, 3)
        .reshape(128, mblocks * kchunks * mb))


def _run(nc, in_maps, trace=False):
    key = ("runner", id(nc))
    if key not in _cache:
        _cache[key] = _make_runner(nc)
    results = _cache[key](in_maps)

    class _R:
        pass

    r = _R()
    r.results = results
    r.exec_time_ns = None
    r.profile_json = None
    return r


def run_scoring(original, posterior, prior, w_router, trace=False):
    of = original.reshape(-1, D)
    df = (posterior.reshape(-1, D) - prior.reshape(-1, D))
    wb = np.ascontiguousarray(np.broadcast_to(w_router, (128, D)),
                              dtype=np.float32)
    in_maps = []
    for c in range(N_CORES):
        sl = slice(c * TOKC, (c + 1) * TOKC)
        in_maps.append(dict(orig=of[sl], dsur=df[sl], wb=wb))
    res = _run(_get("scoring", build_scoring), in_maps, trace)
    out = np.concatenate([res.results[c]["scores"] for c in range(N_CORES)])
    return out.reshape(B, S), res


def prep_decoder_in_maps(sel, pos, gate, ln1_w, wq, wk, wv, wo, ln2_w,
                         w_gate, w_up, w_down):
    F8 = mybir.dt.np(f8)
    r1 = 1.0 / np.sqrt((sel.astype(np.float32) ** 2).mean(-1) + EPS)
    xT_bf = np.ascontiguousarray(sel.T.astype(BF))
    if QKV_FP8:
        # [D, T] -> [128, NK, T] (partition-major k-chunks), fp8
        xT_dev = np.ascontiguousarray(
            sel.T.astype(np.float32).reshape(NK, 128, T).transpose(1, 0, 2)
            .reshape(128, NK * T).astype(F8))
    else:
        xT_dev = xT_bf

    inv_freq = 1.0 / (ROPE_THETA ** (np.arange(0, HD, 2, dtype=np.float32) / HD))
    ang = pos[:, None].astype(np.float32) * inv_freq[None, :]
    cos_v = np.cos(ang).T * r1[None, :]
    sin_v = np.sin(ang).T * r1[None, :]
    cos_t = np.ascontiguousarray(np.concatenate([cos_v, cos_v], 0).astype(BF))
    sin_t = np.ascontiguousarray(np.concatenate([-sin_v, sin_v], 0).astype(BF))
    r1v = r1 / WSCALE if QKV_FP8 else r1
    r1c = np.ascontiguousarray(r1v.reshape(NTOK, 128).T.astype(np.float32))

    wq_f = (ln1_w[:, None] * wq).astype(np.float32)
    wk_f = (ln1_w[:, None] * wk).astype(np.float32)
    wv_f = (ln1_w[:, None] * wv).astype(np.float32)
    wg_f = (ln2_w[:, None] * w_gate).astype(np.float32)
    wu_f = (ln2_w[:, None] * w_up).astype(np.float32)

    if MLP_FP8:
        # combined gate+up: [128, NM, 2, NK, 128] flattened
        def mtile(w):
            return (w * WSCALE).reshape(NK, 128, NM, 128).astype(F8)
        gu = np.stack([mtile(wg_f), mtile(wu_f)], axis=3)  # k,p,m,t,col
        wgu_t = np.ascontiguousarray(
            gu.transpose(1, 2, 3, 0, 4).reshape(128, NM * 2 * NK * 128))
        wg_t = wu_t = None
    else:
        wgu_t = None
        wg_t = _tile_w(wg_f.astype(BF), NK, mblocks=NM)
        wu_t = _tile_w(wu_f.astype(BF), NK, mblocks=NM)
    if MLP_FP8 and DOWN_FP8:
        wd_t = _tile_w((np.asarray(w_down, dtype=np.float32)
                        * WSCALE).astype(F8), NM, mblocks=NK)
    else:
        wd_t = _tile_w(np.asarray(w_down, dtype=np.float32).astype(BF), NM,
                       mblocks=NK)

    qw_dt = F8 if QKV_FP8 else BF
    qw_sc = WSCALE if QKV_FP8 else 1.0
    in_maps = []
    for c in range(N_CORES):
        kvi = c // 2
        im = dict(
            xT=xT_dev,
            xg_in=np.ascontiguousarray(xT_bf[:, c * TG:(c + 1) * TG]),
            wq_in=_tile_w((wq_f[:, c * 256:(c + 1) * 256] * qw_sc)
                          .astype(qw_dt), NK),
            wk_in=_tile_w((wk_f[:, kvi * 128:(kvi + 1) * 128] * qw_sc)
                          .astype(qw_dt), NK),
            wv_in=_tile_w((wv_f[:, kvi * 128:(kvi + 1) * 128] * qw_sc)
                          .astype(qw_dt), NK),
            wo_in=(_tile_w((np.asarray(wo, dtype=np.float32)
                            [c * 256:(c + 1) * 256] * WSCALE).astype(F8), 2)
                   if ATT_FP8 else
                   _tile_w(np.asarray(wo, dtype=np.float32)
                           [c * 256:(c + 1) * 256].astype(BF), 2)),
            wd_in=wd_t,
            cos_in=cos_t, sin_in=sin_t, r1c_in=r1c,
            gate_in=np.ascontiguousarray(np.broadcast_to(
                np.tile(gate[c * TG:(c + 1) * TG], 2).astype(BF),
                (128, 2 * TG))),
        )
        if MLP_FP8:
            im["wgu_in"] = wgu_t
        else:
            im["wg_in"] = wg_t
            im["wu_in"] = wu_t
        in_maps.append(im)
    return in_maps


def run_decoder(sel, pos, gate, ln1_w, wq, wk, wv, wo, ln2_w, w_gate, w_up,
                w_down, trace=False):
    in_maps = prep_decoder_in_maps(sel, pos, gate, ln1_w, wq, wk, wv, wo,
                                   ln2_w, w_gate, w_up, w_down)
    res = _run(_get("decoder", build_decoder), in_maps, trace)
    delta_T = np.concatenate([res.results[c]["delta"] for c in range(N_CORES)],
                             axis=1)
    return np.ascontiguousarray(delta_T.T, dtype=np.float32), res


def _kernel_numpy_fallback(hidden_states, original, posterior, prior,
                           position_ids, w_router, ln1_w, ln2_w, wq, wk, wv,
                           wo, w_gate, w_up, w_down, k):
    """Pure-numpy reference path (used only if shapes diverge from the spec)."""
    x = hidden_states.astype(np.float64)
    scores = (original.astype(np.float64) @ w_router.astype(np.float64)
              + 0.5 * ((posterior.astype(np.float64)
                        - prior.astype(np.float64)) ** 2).mean(-1))
    signal = 1.0 / (1.0 + np.exp(-scores))
    kk = int(k)
    idx = np.sort(np.argpartition(-scores, kk, axis=-1)[:, :kk], axis=-1)
    bidx = np.repeat(np.arange(x.shape[0]), kk)
    tidx = idx.reshape(-1)
    sel = x[bidx, tidx]
    gate = signal[bidx, tidx]
    pos = position_ids[bidx, tidx]
    Tl = sel.shape[0]
    H, KV = 16, 4

    def rms(v, w):
        return v / np.sqrt((v ** 2).mean(-1, keepdims=True) + EPS) * w

    h = rms(sel, ln1_w)
    q = (h @ wq).reshape(Tl, H, HD)
    k_ = (h @ wk).reshape(Tl, KV, HD)
    v_ = (h @ wv).reshape(Tl, KV, HD)
    inv_freq = 1.0 / (ROPE_THETA ** (np.arange(0, HD, 2) / HD))
    angv = pos[:, None] * inv_freq[None, :]
    cos = np.concatenate([np.cos(angv)] * 2, -1)[:, None, :]
    sin = np.concatenate([np.sin(angv)] * 2, -1)[:, None, :]

    def rope(t):
        t1, t2 = np.split(t, 2, -1)
        return t * cos + np.concatenate([-t2, t1], -1) * sin

    q, k_ = rope(q), rope(k_)
    k_ = np.repeat(k_, H // KV, 1)
    v_ = np.repeat(v_, H // KV, 1)
    att = np.einsum("thd,shd->hts", q, k_) / np.sqrt(HD)
    att = np.where(np.tril(np.ones((Tl, Tl), bool))[None], att, -1e9)
    att = np.exp(att - att.max(-1, keepdims=True))
    att /= att.sum(-1, keepdims=True)
    o = np.einsum("hts,shd->thd", att, v_).reshape(Tl, H * HD) @ wo
    x1 = sel + o
    h2 = rms(x1, ln2_w)
    g = h2 @ w_gate
    mlp = (g / (1.0 + np.exp(-g)) * (h2 @ w_up)) @ w_down
    delta = (x1 + mlp - sel) * gate[:, None]
    out = x.copy()
    out[bidx, tidx] += delta
    return out.astype(np.float32)


def kernel(hidden_states, original, posterior, prior, position_ids, w_router,
           ln1_w, ln2_w, wq, wk, wv, wo, w_gate, w_up, w_down, k):
    hidden_states = np.asarray(hidden_states, dtype=np.float32)
    original = np.asarray(original, dtype=np.float32)
    posterior = np.asarray(posterior, dtype=np.float32)
    prior = np.asarray(prior, dtype=np.float32)
    position_ids = np.asarray(position_ids)
    w_router = np.asarray(w_router, dtype=np.float32)
    ln1_w = np.asarray(ln1_w, dtype=np.float32)
    ln2_w = np.asarray(ln2_w, dtype=np.float32)
    wq_, wk_, wv_, wo_ = (np.asarray(a, dtype=np.float32)
                          for a in (wq, wk, wv, wo))
    w_gate_, w_up_, w_down_ = (np.asarray(a, dtype=np.float32)
                               for a in (w_gate, w_up, w_down))
    kk = int(np.asarray(k))

    if (hidden_states.shape != (B, S, D) or kk * B != T):
        return _kernel_numpy_fallback(
            hidden_states, original, posterior, prior, position_ids, w_router,
            ln1_w, ln2_w, wq_, wk_, wv_, wo_, w_gate_, w_up_, w_down_, kk)

    scores, _ = run_scoring(original, posterior, prior, w_router)
    signal = 1.0 / (1.0 + np.exp(-scores.astype(np.float64)))
    idx = np.sort(np.argpartition(-scores, kk, axis=-1)[:, :kk], axis=-1)
    bidx = np.repeat(np.arange(B), kk)
    tidx = idx.reshape(-1)
    sel = np.ascontiguousarray(hidden_states[bidx, tidx])
    gate = signal[bidx, tidx].astype(np.float32)
    pos = position_ids[bidx, tidx]

    delta, _ = run_decoder(sel, pos, gate, ln1_w, wq_, wk_, wv_, wo_,
                           ln2_w, w_gate_, w_up_, w_down_)

    out = hidden_states.copy()
    out[bidx, tidx] += delta
    return out

